# revision 3
# baseline (speedup 1.0000x reference)
"""MoE layer (shared expert + top-2 routed experts) on 8 NeuronCores.

Strategy (expert-parallel, routing-aware):
  - Router (softmax -> top-2 -> renorm) computed on host in float64; it is
    tiny (8192x8) and must match the reference's top-k selection.
  - Core c owns routed expert c: host gathers the tokens routed to expert c
    (~2k of 8192*2 assignments), pads to a common capacity C, and the device
    runs a dense SwiGLU MLP over just those tokens (bf16 matmuls, fp32 accum).
  - The shared expert is data-parallel: core c also runs the shared SwiGLU
    over tokens [c*1024, (c+1)*1024).
  - Combine is done on host: gate-scale each expert's token outputs and
    scatter-add; every token has exactly two routed contributions.

Device layout: activations are kept transposed ([d, tokens]) so the native
[K, M] weight layouts of ew1/ew2/ew3 feed nc.tensor.matmul directly with no
on-device transposes. All matmul inputs are bf16 (PE full rate + FWL),
accumulation is fp32 in PSUM, outputs are written back as bf16 (error
contribution ~3e-4 rel, measured).

DMA regime: all tensors are stored chunk-major in DRAM so every transfer is
one contiguous DMA_DIRECT2D instruction (~0.6us of Sync-engine issue each).
The first token chunk is small (256) and w1 is sliced so the first real
matmul starts as soon as ~1 MiB has landed instead of waiting for the whole
weight prologue.
"""

import sys

for _p in ("/opt/trn_rl_repo",):
    if _p not in sys.path:
        sys.path.append(_p)

import numpy as np
import ml_dtypes

import concourse.bass as bass  # noqa: F401  (engine types via nc)
import concourse.mybir as mybir
import concourse.tile as tile
from concourse import bacc
from concourse.bass_utils import run_bass_kernel_spmd

D = 1024
H = 2048
E = 8
N_TOK = 8192  # 4 * 2048
S = N_TOK // E  # shared-expert tokens per core
KD = D // 128  # 8  k-subtiles over d
KH = H // 128  # 16 k-subtiles over h
MH = H // 128  # 16 m-tiles over h
MD = D // 128  # 8  m-tiles over d
NCHUNK = 512
C0 = 256  # small first chunk: starts the PE as soon as ~1MiB of DMA lands
JW = 4  # w1/w2 column slices (512 cols each)

BF = mybir.dt.bfloat16
F32 = mybir.dt.float32

_program_cache: dict[int, "bacc.Bacc"] = {}


def _chunks_routed(C: int):
    """Chunk sizes for the routed phase: small first chunk, 512s, remainder."""
    sizes = [C0]
    left = C - C0
    while left > NCHUNK:
        sizes.append(NCHUNK)
        left -= NCHUNK
    if left > 0:
        sizes.append(left)
    return sizes


def _chunks_shared():
    return [NCHUNK] * (S // NCHUNK)


def _emit_moe(nc, tc, pools, params, C):
    """Per-core program: routed expert over C gathered tokens, then the
    shared expert over S tokens; SwiGLU MLPs on [d-part, token-free]
    activations. mm3 of each chunk is deferred by one chunk so the PE never
    waits on the DVE h-tile handoff; chunk 0 runs all-u-then-all-v so w2 can
    still be in flight."""
    wpool, xpool, hpool, hspool, opool, ppool, popool = pools
    w1_d, w2_d, w3_d, xg_d, outr_d, s1_d, s2_d, s3_d, xs_d, outs_d = params

    rchunks = _chunks_routed(C)
    schunks = _chunks_shared()

    def wslice(W, k, m):
        # W: [128, J, K, 512]; column m*128 lives in slice j = m//4
        return W[:, m // 4, k, (m % 4) * 128 : (m % 4) * 128 + 128]

    # --- critical-path DMAs first: chunk-0 tokens + leading w1 columns ---
    xc0 = xpool.tile([128, KD, NCHUNK], BF, tag="xc")
    nc.sync.dma_start(xc0[:, :, :C0], xg_d[:, 0, :, :C0])
    w1t = wpool.tile([128, JW, KD, NCHUNK], BF, tag="w1", name="w1")
    nc.sync.dma_start(w1t[:, 0, :, :128], w1_d[:, 0, :, :128])
    nc.sync.dma_start(w1t[:, 0, :, 128:], w1_d[:, 0, :, 128:])
    # PE warmup on an initialized scratch tile while the DMA prologue runs:
    # consumes the HAM cold window so real matmuls start at full clock
    warm = wpool.tile([128, 128], BF, tag="warm")
    nc.vector.memset(warm[:], 0.0)
    pwarm = ppool.tile([128, NCHUNK], F32, tag="pu")
    for _ in range(30):
        nc.tensor.matmul(pwarm[:, :128], warm[:], warm[:], start=True, stop=True)
    # chunk-1 tokens ahead of the bulk weights
    xc1 = xpool.tile([128, KD, NCHUNK], BF, tag="xc")
    nc.sync.dma_start(xc1[:], xg_d[:, 1])
    nc.sync.dma_start(w1t[:, 1], w1_d[:, 1])
    w2t = wpool.tile([128, JW, KD, NCHUNK], BF, tag="w2", name="w2")
    nc.sync.dma_start(w2t[:, 0], w2_d[:, 0])
    nc.sync.dma_start(w1t[:, 2:], w1_d[:, 2:])
    nc.sync.dma_start(w2t[:, 1], w2_d[:, 1])
    nc.sync.dma_start(w2t[:, 2:], w2_d[:, 2:])
    w3t = wpool.tile([128, 2, KH, NCHUNK], BF, tag="w3", name="w3")
    nc.sync.dma_start(w3t[:], w3_d[:])
    # shared-expert w1 has its own SBUF slots -> prefetches during routed phase
    s1t = wpool.tile([128, JW, KD, NCHUNK], BF, tag="s1", name="s1")
    nc.sync.dma_start(s1t[:], s1_d[:])
    s2t = s3t = None

    def emit_uv(W1, W2, xc, h, nsz, split):
        if split:
            for m in range(MH):
                pu = ppool.tile([128, NCHUNK], F32, tag="pu")
                for k in range(KD):
                    nc.tensor.matmul(
                        pu[:, :nsz],
                        wslice(W1, k, m),
                        xc[:, k, :nsz],
                        start=(k == 0),
                        stop=(k == KD - 1),
                    )
                nc.scalar.activation(
                    h[:, m, :nsz], pu[:, :nsz], mybir.ActivationFunctionType.Silu
                )
            for m in range(MH):
                pv = ppool.tile([128, NCHUNK], F32, tag="pv")
                for k in range(KD):
                    nc.tensor.matmul(
                        pv[:, :nsz],
                        wslice(W2, k, m),
                        xc[:, k, :nsz],
                        start=(k == 0),
                        stop=(k == KD - 1),
                    )
                nc.vector.tensor_mul(h[:, m, :nsz], h[:, m, :nsz], pv[:, :nsz])
        else:
            for m in range(MH):
                pu = ppool.tile([128, NCHUNK], F32, tag="pu")
                pv = ppool.tile([128, NCHUNK], F32, tag="pv")
                for k in range(KD):
                    nc.tensor.matmul(
                        pu[:, :nsz],
                        wslice(W1, k, m),
                        xc[:, k, :nsz],
                        start=(k == 0),
                        stop=(k == KD - 1),
                    )
                for k in range(KD):
                    nc.tensor.matmul(
                        pv[:, :nsz],
                        wslice(W2, k, m),
                        xc[:, k, :nsz],
                        start=(k == 0),
                        stop=(k == KD - 1),
                    )
                hs = hspool.tile([128, NCHUNK], F32, tag="hs")
                nc.scalar.activation(
                    hs[:, :nsz], pu[:, :nsz], mybir.ActivationFunctionType.Silu
                )
                nc.vector.tensor_mul(h[:, m, :nsz], hs[:, :nsz], pv[:, :nsz])

    def emit_mm3(W3, h, nsz, slot, out_d, last):
        ot = opool.tile([128, MD, NCHUNK], BF, tag="ot")
        for mo in range(MD):
            po = popool.tile([128, NCHUNK], F32, tag="po")
            for k in range(KH):
                nc.tensor.matmul(
                    po[:, :nsz],
                    wslice(W3, k, mo),
                    h[:, k, :nsz],
                    start=(k == 0),
                    stop=(k == KH - 1),
                )
            nc.vector.tensor_copy(ot[:, mo, :nsz], po[:, :nsz])
        if last:
            # split the final writeback so the drain after the last matmul
            # is one small transfer, not a 2 MiB one
            nc.sync.dma_start(out_d[:, slot, : MD - 1, :nsz], ot[:, : MD - 1, :nsz])
            nc.sync.dma_start(out_d[:, slot, MD - 1 :, :nsz], ot[:, MD - 1 :, :nsz])
        else:
            nc.sync.dma_start(out_d[:, slot, :, :nsz], ot[:, :, :nsz])

    jobs = [("r", si, sz) for si, sz in enumerate(rchunks)]
    jobs += [("s", si, sz) for si, sz in enumerate(schunks)]

    deferred = None
    for ji, (ph, slot, nsz) in enumerate(jobs):
        if ph == "r":
            x_d, out_d, W1, W2 = xg_d, outr_d, w1t, w2t
        else:
            if s2t is None:
                s2t = w2t  # reuses w2 slots (WAR-ordered by the scheduler)
                nc.sync.dma_start(s2t[:, :2], s2_d[:, :2])
                nc.sync.dma_start(s2t[:, 2:], s2_d[:, 2:])
            x_d, out_d, W1, W2 = xs_d, outs_d, s1t, s2t
        if ji == 0:
            xc = xc0
        elif ji == 1:
            xc = xc1
        else:
            xc = xpool.tile([128, KD, NCHUNK], BF, tag="xc")
            nc.sync.dma_start(xc[:, :, :nsz], x_d[:, slot, :, :nsz])
        h = hpool.tile([128, KH, NCHUNK], BF, tag="h")
        emit_uv(W1, W2, xc, h, nsz, split=(ji == 0))
        if deferred is not None:
            emit_mm3(*deferred, last=False)
            if ph == "s" and s3t is None:
                s3t = w3t  # reuses w3 slots (WAR-ordered)
                nc.sync.dma_start(s3t[:], s3_d[:])
        deferred = ((w3t if ph == "r" else s3t), h, nsz, slot, out_d)
    emit_mm3(*deferred, last=True)


def _build_program(C: int):
    nc = bacc.Bacc(None, target_bir_lowering=False)

    nr = len(_chunks_routed(C))
    ns = len(_chunks_shared())
    xg_d = nc.declare_dram_parameter("xg", [128, nr, KD, NCHUNK], BF, isOutput=False)
    w1_d = nc.declare_dram_parameter("w1", [128, JW, KD, NCHUNK], BF, isOutput=False)
    w2_d = nc.declare_dram_parameter("w2", [128, JW, KD, NCHUNK], BF, isOutput=False)
    w3_d = nc.declare_dram_parameter("w3", [128, 2, KH, NCHUNK], BF, isOutput=False)
    xs_d = nc.declare_dram_parameter("xs", [128, ns, KD, NCHUNK], BF, isOutput=False)
    s1_d = nc.declare_dram_parameter("s1", [128, JW, KD, NCHUNK], BF, isOutput=False)
    s2_d = nc.declare_dram_parameter("s2", [128, JW, KD, NCHUNK], BF, isOutput=False)
    s3_d = nc.declare_dram_parameter("s3", [128, 2, KH, NCHUNK], BF, isOutput=False)
    outr_d = nc.declare_dram_parameter("out_r", [128, nr, MD, NCHUNK], BF, isOutput=True)
    outs_d = nc.declare_dram_parameter("out_s", [128, ns, MD, NCHUNK], BF, isOutput=True)

    with tile.TileContext(nc) as tc:
        with (
            tc.tile_pool(name="wpool", bufs=1) as wpool,
            tc.tile_pool(name="xpool", bufs=2) as xpool,
            tc.tile_pool(name="hpool", bufs=2) as hpool,
            tc.tile_pool(name="hspool", bufs=3) as hspool,
            tc.tile_pool(name="opool", bufs=2) as opool,
            tc.tile_pool(name="ppool", bufs=3, space="PSUM") as ppool,
            tc.tile_pool(name="popool", bufs=2, space="PSUM") as popool,
        ):
            pools = (wpool, xpool, hpool, hspool, opool, ppool, popool)
            params = (
                w1_d, w2_d, w3_d, xg_d, outr_d,
                s1_d, s2_d, s3_d, xs_d, outs_d,
            )
            _emit_moe(nc, tc, pools, params, C)

    nc.compile()
    return nc


def _get_program(C: int):
    if C not in _program_cache:
        _program_cache[C] = _build_program(C)
    return _program_cache[C]


def _to_dev_layout(a: np.ndarray) -> np.ndarray:
    """[T, d_in] host activation -> [128, d_in//128, T] bf16 device layout
    (d_in on partitions as d = po*128 + pi, T on the free dim)."""
    t, din = a.shape
    b = a.T.reshape(din // 128, 128, t).transpose(1, 0, 2)
    return np.ascontiguousarray(b.astype(ml_dtypes.bfloat16))


def _chunked_x(a: np.ndarray, sizes) -> np.ndarray:
    """[T, d_in] host activation -> [128, nslots, d_in//128, 512] bf16,
    slot si holding tokens sum(sizes[:si]) .. +sizes[si] in cols [0, size)."""
    dev = _to_dev_layout(a)  # [128, KD, T]
    out = np.zeros((128, len(sizes), dev.shape[1], NCHUNK), dtype=ml_dtypes.bfloat16)
    n0 = 0
    for si, sz in enumerate(sizes):
        out[:, si, :, :sz] = dev[:, :, n0 : n0 + sz]
        n0 += sz
    return np.ascontiguousarray(out)


def _w_sliced(a: np.ndarray, j: int) -> np.ndarray:
    """[K, M] host weight -> [128, j, K//128, 512] bf16: K on partitions,
    M split into j slices of 512 columns."""
    k, m = a.shape
    dev = _to_dev_layout(a.T)  # [128, K//128, M]
    b = dev.reshape(128, k // 128, j, NCHUNK).transpose(0, 2, 1, 3)
    return np.ascontiguousarray(b)


def _unchunk_out(a: np.ndarray, sizes) -> np.ndarray:
    """[128, nslots, MD, 512] bf16 device output -> [T, 1024] fp32."""
    total = sum(sizes)
    out = np.empty((total, MD * 128), dtype=np.float32)
    n0 = 0
    for si, sz in enumerate(sizes):
        blk = a[:, si, :, :sz].astype(np.float32)  # [128, MD, sz]
        out[n0 : n0 + sz] = blk.transpose(1, 0, 2).reshape(MD * 128, sz).T
        n0 += sz
    return out


def kernel(x, sw1, sw2, sw3, ew1, ew2, ew3, rw, rb):
    x = np.asarray(x, dtype=np.float32)
    sw1, sw2, sw3 = (np.asarray(a, dtype=np.float32) for a in (sw1, sw2, sw3))
    ew1, ew2, ew3 = (np.asarray(a, dtype=np.float32) for a in (ew1, ew2, ew3))
    rw = np.asarray(rw, dtype=np.float32)
    rb = np.asarray(rb, dtype=np.float32)
    xf = np.ascontiguousarray(x.reshape(N_TOK, D), dtype=np.float32)

    # --- host router (float64 to track the fp32 reference's ordering) ---
    logits = xf.astype(np.float64) @ rw.astype(np.float64) + rb.astype(np.float64)
    logits -= logits.max(axis=1, keepdims=True)
    p = np.exp(logits)
    p /= p.sum(axis=1, keepdims=True)
    order = np.argsort(-p, axis=1, kind="stable")
    idx = order[:, :2]  # [N, 2] expert ids, top-2
    w = np.take_along_axis(p, idx, axis=1)
    w = w / w.sum(axis=1, keepdims=True)

    tok_lists = []
    gate_lists = []
    for e in range(E):
        sel = idx == e  # [N, 2]
        any_e = sel.any(axis=1)
        tok = np.nonzero(any_e)[0]
        ge = np.where(sel[tok, 0], w[tok, 0], w[tok, 1])
        tok_lists.append(tok)
        gate_lists.append(ge.astype(np.float64))

    maxT = max(len(t) for t in tok_lists)
    C = max(C0 + 1, maxT)
    rchunks = _chunks_routed(C)
    schunks = _chunks_shared()

    nc = _get_program(C)

    s1 = _w_sliced(sw1, JW)
    s2 = _w_sliced(sw2, JW)
    s3 = _w_sliced(sw3, 2)

    in_maps = []
    for e in range(E):
        tok = tok_lists[e]
        xg = np.zeros((C, D), dtype=np.float32)
        xg[: len(tok)] = xf[tok]
        in_maps.append(
            {
                "xg": _chunked_x(xg, rchunks),
                "w1": _w_sliced(ew1[e], JW),
                "w2": _w_sliced(ew2[e], JW),
                "w3": _w_sliced(ew3[e], 2),
                "xs": _chunked_x(xf[e * S : (e + 1) * S], schunks),
                "s1": s1,
                "s2": s2,
                "s3": s3,
            }
        )

    res = run_bass_kernel_spmd(nc, in_maps, list(range(E)))

    # --- host combine: shared shards + gated scatter-add of routed outputs ---
    out = np.empty((N_TOK, D), dtype=np.float32)
    for e in range(E):
        out[e * S : (e + 1) * S] = _unchunk_out(res.results[e]["out_s"], schunks)

    all_tok = np.concatenate(tok_lists)
    all_contrib = np.concatenate(
        [
            _unchunk_out(res.results[e]["out_r"], rchunks)[: len(tok_lists[e])]
            * gate_lists[e][:, None].astype(np.float32)
            for e in range(E)
        ]
    )
    pos = np.argsort(all_tok, kind="stable")
    # every token has exactly two routed contributions (top-2 routing)
    out += all_contrib[pos[0::2]]
    out += all_contrib[pos[1::2]]

    return out.reshape(x.shape).astype(np.float32)
